# revision 34
# baseline (speedup 1.0000x reference)
"""DeepseekV3 MLA forward on 8 TRN2 NeuronCores.

Sharding: data-parallel over batch (B=2 -> 2 groups of 4 cores), tensor-
parallel over heads within each batch group (32 heads -> 4 groups of 8).
Each core computes its batch element's full latent projections (wq_a /
wkv_a replicated), its 8 heads' q/k/v expansions + attention, and a
partial output projection (wo row-shard); the host sums the 4 partial
outputs per batch element.

Precision strategy (rel-err budget 2e-2; measured ~2e-3):
  - Stage A (x @ wq_a / wkv_a) runs in SPLIT fp8e4m3 with DoubleRow
    matmuls: both operands are host tensors, so the host provides
    hi = Q(s*v) and lo = Q(s*v - hi) at the SAME scale. The product is
    hh + hl + lh (lo*lo dropped) = 3 slot-products = 1.5 DoubleRow
    instructions per pair of 128-contraction tiles -> 0.75 cycles/row
    vs 1.0 for f32r, with ~bf16-level accuracy.
  - rms sum-of-squares: fp8 squares + fp8-DoubleRow ones-matmul.
  - Output projection (attn @ wo) in split fp8 as well: attnF = 16*attn
    (f32) -> hi (ACT copy) + lo (DVE subtract), wo hi/lo from host.
    PSUM result = 4096*out, DMA'd straight from PSUM to DRAM; the host
    descales by 1/4096 while summing the 4 partials.
  - Everything else (q/kv up-proj, scores, softmax, attn*v) stays f32r.
  - RMSNorm weights are folded into wq_b/wkv_b rows on the host.

Dataflow on device keeps activations transposed ([feature, token]) so
every matmul contracts over the partition dim with no on-device
transposes anywhere (see per-stage comments).  The softmax denominator
uses a ones/16-matmul so attnF lands pre-scaled by 16 for fp8.
All f32r matmuls are FP22-truncated fp32, single pass.
"""

import os
import sys

import numpy as np

sys.path.insert(0, "/opt/trn_rl_repo")

B, T, HID = 2, 1024, 4096
H, D_NOPE, D_ROPE, D_V = 32, 128, 64, 128
D_QK = D_NOPE + D_ROPE
Q_RANK, KV_RANK = 1536, 512
THETA, EPS = 10000.0, 1e-6
SCALE = float(D_QK) ** -0.5
NMASK = -30000.0

HG = H // 4          # heads per core = 8
P = 128
QCH = Q_RANK // P    # 12 latent chunks (q)
KCH = KV_RANK // P   # 4 latent chunks (kv)
KP = HID // 256      # 16 contraction pairs for stage A
TQ = 512             # token tile (free dim) for most matmuls
NT = T // TQ         # 2 token tiles
TC = T // P          # 8 token chunks of 128
NHID = HID // TQ     # 8 output column tiles

SX = 32.0            # x fp8 scale
SWA = 2048.0         # wq_a/wkv_a fp8 scale
LS = SX * SWA        # stage-A psum scale
SAT = 16.0           # attn fp8 scale (from ones=1/16 denominator)
SWO = 256.0          # wo fp8 scale
OSC = SAT * SWO      # output psum descale (host side)

_CACHED = {}
STAGE_MARKS = []


def _build_program():
    import contextlib

    import concourse.bacc as bacc
    import concourse.mybir as mybir
    import concourse.tile as tile

    f32 = mybir.dt.float32
    f32r = mybir.dt.float32r
    bf16 = mybir.dt.bfloat16
    f8 = mybir.dt.float8e4
    AF = mybir.ActivationFunctionType
    ALU = mybir.AluOpType
    DR = mybir.MatmulPerfMode.DoubleRow

    nc = bacc.Bacc()

    # ---- DRAM I/O (per-core shapes; SPMD across the 8 cores) ----
    # stage-A operands host-split into same-scale fp8 hi/lo pairs, packed
    # partition-major with the DoubleRow slot dim adjacent
    xhl = nc.dram_tensor("xhl", (2, 4, P, 4, 2, T), f8, kind="ExternalInput")
    wq8 = nc.dram_tensor("wq8", (QCH, P, 2, KP, 2, P), f8, kind="ExternalInput")
    wkv8 = nc.dram_tensor("wkv8", (KCH, P, 2, KP, 2, P), f8, kind="ExternalInput")
    wr8 = nc.dram_tensor("wr8", (P, 2, KP, 2, 64), f8, kind="ExternalInput")
    wqbn = nc.dram_tensor("wqbn", (HG, P, QCH, P), mybir.dt.bfloat16, kind="ExternalInput")
    wqbr = nc.dram_tensor("wqbr", (HG // 2, P, QCH, P), mybir.dt.bfloat16, kind="ExternalInput")
    wkbn = nc.dram_tensor("wkbn", (P, HG, KCH, P), mybir.dt.bfloat16, kind="ExternalInput")
    wkbv = nc.dram_tensor("wkbv", (2, P, KCH, TQ), mybir.dt.bfloat16, kind="ExternalInput")
    wo8 = nc.dram_tensor("wo8", (NHID, P, 2, 4, 2, TQ), f8, kind="ExternalInput")
    cos4 = nc.dram_tensor("cos4", (P, T), mybir.dt.bfloat16, kind="ExternalInput")
    sin4 = nc.dram_tensor("sin4", (P, T), mybir.dt.bfloat16, kind="ExternalInput")  # +-sin
    cmask = nc.dram_tensor("cmask", (P, 7 * P), f32, kind="ExternalInput")
    kbias = nc.dram_tensor("kbias", (P, TC), f32, kind="ExternalInput")
    onesd = nc.dram_tensor("onesd", (P, P), mybir.dt.bfloat16, kind="ExternalInput")  # 1/16
    ones8d = nc.dram_tensor("ones8d", (P, 2 * P), f8, kind="ExternalInput")
    out = nc.dram_tensor("out", (NT, NHID, 4, P, TQ), mybir.dt.bfloat16, kind="ExternalOutput")

    def r(ap):
        return ap.bitcast(f32r)

    with tile.TileContext(nc) as tc, contextlib.ExitStack() as rstack:
        with (
            tc.tile_pool(name="const", bufs=1) as const,
            tc.tile_pool(name="psmm", bufs=3, space="PSUM") as psum,
            tc.tile_pool(name="pspd", bufs=2, space="PSUM") as pspd,
            tc.tile_pool(name="pssc", bufs=3, space="PSUM") as pssc,
        ):
            # ---- constants (persistent; DMAs deferred past the first
            # stage-A tiles so they don't delay the first matmuls) ----
            ones_sb = const.tile([P, P], bf16, tag="ones")      # value 1/16
            ones8_sb = const.tile([P, 2, P], f8, tag="ones8")   # value 1/16
            cos_sb = const.tile([P, T], bf16, tag="cos")
            sin_sb = const.tile([P, T], bf16, tag="sin")
            kb_sb = const.tile([P, TC], f32, tag="kb")
            zero_b = const.tile([P, 1], f32, tag="zb")
            nc.vector.memset(zero_b[:], 0.0)
            eps_b = const.tile([P, 1], f32, tag="eb")
            nc.vector.memset(eps_b[:], EPS)

            def emit_const_dmas():
                nc.sync.dma_start(ones_sb[:], onesd[:, :])
                nc.sync.dma_start(ones8_sb[:], ones8d[:, :])
                nc.sync.dma_start(cos_sb[:], cos4[:, :])
                nc.sync.dma_start(sin_sb[:], sin4[:, :])
                nc.sync.dma_start(kb_sb[:], kbias[:, :])

            def rmsnorm(lat, nch, fan, sspool, sstag, wrk):
                # fp8 squares (scale 2 -> (2*lat)^2 <= ~121) + fp8-DR
                # ones/16 matmul: pd = sum(lat^2)/4
                for t in range(NT):
                    ssp = sspool.tile([P, TQ], f32, tag=sstag, name="ssp")
                    npr = nch // 2
                    for pr in range(npr):
                        sq = wrk.tile([P, 2, TQ], f8, tag="sq", name="sq")
                        for s in range(2):
                            nc.scalar.activation(
                                sq[:, s, :],
                                lat[2 * pr + s][:, t * TQ : (t + 1) * TQ],
                                AF.Square,
                                bias=zero_b[:],
                                scale=2.0,
                            )
                        nc.tensor.matmul(
                            ssp[:],
                            ones8_sb[:],
                            sq[:],
                            start=(pr == 0),
                            stop=(pr == npr - 1),
                            perf_mode=DR,
                        )
                    std = wrk.tile([P, TQ], f32, tag="std", name="std")
                    nc.scalar.activation(
                        std[:], ssp[:], AF.Sqrt, bias=eps_b[:], scale=4.0 / fan
                    )
                    rstd = wrk.tile([P, TQ], bf16, tag="rstd", name="rstd")
                    with nc.allow_low_precision("rmsnorm rstd"):
                        nc.vector.reciprocal(rstd[:], std[:])
                    for m in range(nch):
                        sl = lat[m][:, t * TQ : (t + 1) * TQ]
                        nc.gpsimd.tensor_tensor(sl, sl, rstd[:], ALU.mult)

            with tc.tile_pool(name="qlatp", bufs=1) as qlatp:
                qlat = [
                    qlatp.tile([P, T], bf16, tag=f"qlat{i}", name=f"qlat{i}")
                    for i in range(QCH)
                ]
                with tc.tile_pool(name="kvlatp", bufs=1) as kvlatp:
                    kvlat = [
                        kvlatp.tile([P, T], bf16, tag=f"kvlat{i}", name=f"kvlat{i}")
                        for i in range(KCH + 1)
                    ]
                    with (
                        tc.tile_pool(name="wkbnp", bufs=1) as wkbnp,
                        tc.tile_pool(name="wkbvp", bufs=1) as wkbvp,
                    ):
                        # stage-D weights: pool reserved up front; DMAs
                        # emitted mid-stage-A
                        wkn = wkbnp.tile(
                            [P, HG, KCH, P], bf16, tag="wkbn", name="wkbn"
                        )
                        wkvts = [
                            wkbvp.tile(
                                [P, KCH, TQ], bf16, tag=f"wkbv{quad}", name="wkbv"
                            )
                            for quad in range(2)
                        ]

                        def emit_dweight_dmas():
                            nc.sync.dma_start(wkn[:], wkbn[:, :, :, :])
                            for quad in range(2):
                                nc.sync.dma_start(
                                    wkvts[quad][:],
                                    wkbv[quad, :, :, :],
                                )

                        # kpe2 lives to the end (right side)
                        kpep = rstack.enter_context(
                            tc.tile_pool(name="kpep", bufs=1, side="right"))
                        kpe2 = kpep.tile([P, T], bf16, tag="kpe2")

                        # ---- stage A: latent projections, split fp8 ----
                        # mblocks: (dram, chunk index or None for rope,
                        #           width, dest tile)
                        mblocks = (
                            [(wkv8, m, P, kvlat[m]) for m in range(KCH)]
                            + [(wr8, None, 64, kvlat[KCH])]
                            + [(wq8, m, P, qlat[m]) for m in range(QCH)]
                        )

                        wrkA_cm = tc.tile_pool(name="wrkA", bufs=2)
                        wrkA = wrkA_cm.__enter__()
                        kswp_cm = tc.tile_pool(name="kswp", bufs=1)
                        kswp = kswp_cm.__enter__()
                        with (
                            tc.tile_pool(name="xk", bufs=8) as xkp,
                            tc.tile_pool(name="wA", bufs=3) as wAp,
                        ):
                            # first weight chunk DMA'd before the x stream
                            # so PE can start as soon as x group 0 lands
                            wt0 = wAp.tile([P, 2, KP, 2, P], f8, tag="wA",
                                           name="wA")
                            nc.sync.dma_start(
                                wt0[:], mblocks[0][0][mblocks[0][1],
                                                      :, :, :, :, :])
                            # first 3 chunks interleaved x-group-major so
                            # PE isn't starved while the x stream lands
                            NPRE = 3
                            pre_w = [wt0]
                            for mbi in range(1, NPRE):
                                wdram, blki, mw, dest = mblocks[mbi]
                                wt = wAp.tile(
                                    [P, 2, KP, 2, P], f8, tag="wA", name="wA")
                                nc.sync.dma_start(
                                    wt[:], wdram[blki, :, :, :, :, :])
                                pre_w.append(wt)
                            # x hi/lo tiles: 4 pairs per DMA so the HWDGE
                            # issue cost doesn't delay the first weights
                            xg = [[None] * 4, [None] * 4]
                            for hl in range(2):
                                for g in range(4):
                                    xt_ = xkp.tile(
                                        [P, 4, 2, T], f8, tag="xk", name="xk"
                                    )
                                    nc.sync.dma_start(
                                        xt_[:], xhl[hl, g, :, :, :, :]
                                    )
                                    xg[hl][g] = xt_
                            pre_ps = {}
                            pre_pool = [psum, pssc, pspd]
                            pre_tag = ["mm", "psc", "pd"]
                            for mbi in range(NPRE):
                                for t in range(NT):
                                    pre_ps[(mbi, t)] = pre_pool[mbi].tile(
                                        [P, TQ], f32, tag=pre_tag[mbi],
                                        name="psA")
                            for g in range(4):
                                for mbi in range(NPRE):
                                    for t in range(NT):
                                        ps_ = pre_ps[(mbi, t)]
                                        tsl = slice(t * TQ, (t + 1) * TQ)
                                        for j in range(4):
                                            pair = g * 4 + j
                                            for pi, (whl, xhl_) in enumerate((
                                                (0, 0), (1, 0), (0, 1),
                                            )):
                                                nc.tensor.matmul(
                                                    ps_[:, :],
                                                    pre_w[mbi][:, whl, pair,
                                                               :, :],
                                                    xg[xhl_][g][:, j, :, tsl],
                                                    start=(pair == 0
                                                           and pi == 0),
                                                    stop=(pair == KP - 1
                                                          and pi == 2),
                                                    perf_mode=DR,
                                                )
                            for mbi in range(NPRE):
                                wdram, blki, mw, dest = mblocks[mbi]
                                for t in range(NT):
                                    tsl = slice(t * TQ, (t + 1) * TQ)
                                    nc.scalar.activation(
                                        dest[:, tsl],
                                        pre_ps[(mbi, t)][:],
                                        AF.Copy,
                                        scale=1.0 / LS,
                                    )
                            emit_const_dmas()
                            for mbi, (wdram, blki, mw, dest) in \
                                    enumerate(mblocks):
                                if mbi < NPRE:
                                    continue
                                if mbi == NPRE:
                                    emit_dweight_dmas()
                                wt = wAp.tile(
                                    [P, 2, KP, 2, P], f8, tag="wA",
                                    name="wA",
                                )
                                if blki is None:
                                    nc.sync.dma_start(
                                        wt[:, :, :, :, :mw],
                                        wdram[:, :, :, :, :])
                                else:
                                    nc.sync.dma_start(
                                        wt[:], wdram[blki, :, :, :, :, :])
                                for t in range(NT):
                                    pp, ptag = (
                                        (psum, "mm")
                                        if (mbi + t) % 2 == 0
                                        else (pssc, "psc")
                                    )
                                    ps_ = pp.tile(
                                        [P, TQ], f32, tag=ptag, name="psA"
                                    )
                                    tsl = slice(t * TQ, (t + 1) * TQ)
                                    for pair in range(KP):
                                        g, j = pair // 4, pair % 4
                                        for pi, (whl, xhl_) in enumerate((
                                            (0, 0), (1, 0), (0, 1),
                                        )):
                                            nc.tensor.matmul(
                                                ps_[:mw, :],
                                                wt[:, whl, pair, :, :mw],
                                                xg[xhl_][g][:, j, :, tsl],
                                                start=(pair == 0 and pi == 0),
                                                stop=(pair == KP - 1
                                                      and pi == 2),
                                                perf_mode=DR,
                                            )
                                    nc.scalar.activation(
                                        dest[:mw, tsl],
                                        ps_[:mw, :],
                                        AF.Copy,
                                        scale=1.0 / LS,
                                    )
                                if mbi == KCH:
                                    # kv latents + rope chunk done: norm +
                                    # k rope now, overlapping the q blocks
                                    STAGE_MARKS.append(("A2kv", nc.next_id()))
                                    rmsnorm(kvlat, KCH, KV_RANK,
                                            pspd, "pd", wrkA)
                                    ksw = kswp.tile(
                                        [P, T], bf16, tag="ksw", name="ksw"
                                    )
                                    kswf = ksw[:]
                                    nc.sync.dma_start(
                                        kpe2[0:64, :], kvlat[KCH][0:64, :])
                                    nc.sync.dma_start(
                                        kpe2[64:128, :], kvlat[KCH][0:64, :])
                                    nc.sync.dma_start(
                                        ksw[0:32, :], kvlat[KCH][32:64, :])
                                    nc.sync.dma_start(
                                        ksw[32:64, :], kvlat[KCH][0:32, :])
                                    nc.sync.dma_start(
                                        ksw[64:96, :], kvlat[KCH][32:64, :])
                                    nc.sync.dma_start(
                                        ksw[96:128, :], kvlat[KCH][0:32, :])
                                    nc.vector.tensor_tensor(
                                        kswf, kswf, sin_sb[:], ALU.mult)
                                    nc.vector.tensor_tensor(
                                        kpe2[:], kpe2[:], cos_sb[:], ALU.mult)
                                    nc.vector.tensor_tensor(
                                        kpe2[:], kpe2[:], kswf, ALU.add)

                        # x + stage-A weight pools closed: SBUF freed for
                        # the kTn / vq activation tiles
                        wqbp_e = rstack.enter_context(
                            tc.tile_pool(name="wqbpe", bufs=4, side="right"))
                        wqb_pre = {}
                        for p__ in range(3):
                            wt_ = wqbp_e.tile(
                                [P, QCH, P], bf16, tag="wqb", name="wqbr")
                            nc.sync.dma_start(wt_[:], wqbr[p__, :, :, :])
                            wqb_pre[p__] = wt_
                        kTnp = rstack.enter_context(
                            tc.tile_pool(name="kTnp", bufs=1, side="right"))
                        kTn = kTnp.tile([P, HG, T], bf16, tag="kTn")
                        vqp = rstack.enter_context(
                            tc.tile_pool(name="vqp", bufs=2, side="right"))
                        vq = [
                            vqp.tile([P, TC, 4 * D_V], bf16, tag="vq", name="vq")
                            for _ in range(2)
                        ]

                        STAGE_MARKS.append(("A2q", nc.next_id()))
                        rmsnorm(qlat, QCH, Q_RANK, pssc, "psc", wrkA)
                        kswp_cm.__exit__(None, None, None)
                        wrkA_cm.__exit__(None, None, None)
                        STAGE_MARKS.append(("D", nc.next_id()))
                        # ---- stage D: kT_nope per head, v per quad ----
                        for h in range(HG):
                            pp, ptag = (psum, "mm") if h % 2 == 0 else (pssc, "psc")
                            pst = [
                                pp.tile([P, TQ], f32, tag=ptag, name="psD")
                                for _ in range(NT)
                            ]
                            for k in range(KCH):
                                for t in range(NT):
                                    nc.tensor.matmul(
                                        pst[t][:],
                                        wkn[:, h, k, :],
                                        kvlat[k][:, t * TQ : (t + 1) * TQ],
                                        start=(k == 0),
                                        stop=(k == KCH - 1),
                                    )
                            for t in range(NT):
                                nc.scalar.copy(
                                    kTn[:, h, t * TQ : (t + 1) * TQ], pst[t][:]
                                )
                        for quad in range(2):
                            for tkc in range(TC):
                                pp, ptag = (
                                    (psum, "mm") if tkc % 2 == 0 else (pssc, "psc")
                                )
                                ps_ = pp.tile([P, TQ], f32, tag=ptag, name="psV")
                                for k in range(KCH):
                                    nc.tensor.matmul(
                                        ps_[:],
                                        kvlat[k][:, tkc * P : (tkc + 1) * P],
                                        wkvts[quad][:, k, :],
                                        start=(k == 0),
                                        stop=(k == KCH - 1),
                                    )
                                if tkc % 2 == 0:
                                    nc.vector.tensor_copy(
                                        vq[quad][:, tkc, :], ps_[:])
                                else:
                                    nc.scalar.copy(
                                        vq[quad][:, tkc, :], ps_[:])


                # kvlat + stage-D weight pools closed here
                actq = rstack.enter_context(
                    tc.tile_pool(name="actq", bufs=1, side="right"))
                qTn = actq.tile([P, HG, T], bf16, tag="qTn")
                qTr = actq.tile([P, HG // 2, T], bf16, tag="qTr")

                cmp_ = rstack.enter_context(
                    tc.tile_pool(name="cmp", bufs=1, side="right"))
                cm_sb = cmp_.tile([P, 7 * P], f32, tag="cm")
                nc.sync.dma_start(cm_sb[:], cmask[:, :])
                STAGE_MARKS.append(("B", nc.next_id()))
                # ---- stage B: qT per head (one DMA per head/pair) ----
                with (
                    tc.tile_pool(name="qswp", bufs=1) as qswp,
                ):
                    wqbp = wqbp_e
                    for p_ in range(HG // 2):
                        if p_ in wqb_pre:
                            wt = wqb_pre[p_]
                        else:
                            wt = wqbp.tile(
                                [P, QCH, P], bf16, tag="wqb", name="wqbr")
                            nc.sync.dma_start(wt[:], wqbr[p_, :, :, :])
                        pp, ptag = (psum, "mm") if p_ % 2 == 0 else (pssc, "psc")
                        pst = [
                            pp.tile([P, TQ], f32, tag=ptag, name="psB2")
                            for _ in range(NT)
                        ]
                        for k in range(QCH):
                            for t in range(NT):
                                nc.tensor.matmul(
                                    pst[t][:],
                                    wt[:, k, :],
                                    qlat[k][:, t * TQ : (t + 1) * TQ],
                                    start=(k == 0),
                                    stop=(k == QCH - 1),
                                )
                        for t in range(NT):
                            nc.scalar.copy(
                                qTr[:, p_, t * TQ : (t + 1) * TQ], pst[t][:]
                            )
                        # rope this pair immediately (overlaps next pair)
                        qsw = qswp.tile([P, T], bf16, tag="qsw", name="qsw")
                        qp = qTr[:, p_, :]
                        nc.sync.dma_start(qsw[0:32, :], qp[32:64, :])
                        nc.sync.dma_start(qsw[32:64, :], qp[0:32, :])
                        nc.sync.dma_start(qsw[64:96, :], qp[96:128, :])
                        nc.sync.dma_start(qsw[96:128, :], qp[64:96, :])
                        nc.vector.tensor_tensor(qsw[:], qsw[:], sin_sb[:], ALU.mult)
                        nc.vector.tensor_tensor(qp, qp, cos_sb[:], ALU.mult)
                        nc.vector.tensor_tensor(qp, qp, qsw[:], ALU.add)
                    for h in range(HG):
                        wt = wqbp.tile(
                            [P, QCH, P], bf16, tag="wqb", name="wqbn")
                        nc.sync.dma_start(wt[:], wqbn[h, :, :, :])
                        pp, ptag = (psum, "mm") if h % 2 == 0 else (pssc, "psc")
                        pst = [
                            pp.tile([P, TQ], f32, tag=ptag, name="psB")
                            for _ in range(NT)
                        ]
                        for k in range(QCH):
                            for t in range(NT):
                                nc.tensor.matmul(
                                    pst[t][:],
                                    wt[:, k, :],
                                    qlat[k][:, t * TQ : (t + 1) * TQ],
                                    start=(k == 0),
                                    stop=(k == QCH - 1),
                                )
                        for t in range(NT):
                            nc.scalar.copy(
                                qTn[:, h, t * TQ : (t + 1) * TQ], pst[t][:]
                            )
            STAGE_MARKS.append(("EF", nc.next_id()))
            # kvlat + qlat pools closed here
            # ---- stages E+F per token tile (t=1 first: its leading tk
            # chunks need no causal mask, hiding the mask DMA) ----
            with (
                tc.tile_pool(name="attp", bufs=1) as attp,
                tc.tile_pool(name="wrkE", bufs=3) as wrkE,
                tc.tile_pool(name="recp", bufs=2) as recp,
                tc.tile_pool(name="afp", bufs=3) as afp,
                tc.tile_pool(name="wop", bufs=3) as wop,
                tc.tile_pool(name="outs", bufs=4) as outp,
            ):
                a8 = {}
                for t in (1, 0):
                    # attn in split fp8: hi/lo tiles [dv, head, tq]
                    a8[t] = (
                        attp.tile([P, HG, TQ], f8, tag=f"a8h{t}", name="a8h"),
                        attp.tile([P, HG, TQ], f8, tag=f"a8l{t}", name="a8l"),
                    )

                def attn_head(t, h):
                    # 256-query sub-tiling: query halves ua=2t, ub=2t+1.
                    # Key chunks 0..4t+1 are shared (both halves, 512-wide
                    # matmuls); chunks 4t+2, 4t+3 touch only ub (256-wide).
                    # Masked work drops ~25% vs 512-query tiling.
                    a8h, a8l = a8[t]
                    hb = 64 * (h % 2)
                    qsl = slice(t * TQ, (t + 1) * TQ)
                    qslb = slice(t * TQ + 256, (t + 1) * TQ)
                    nsh = 4 * t + 2
                    exs = []
                    for c in range(nsh + 2):
                        shared = c < nsh
                        w_ps = TQ if shared else 256
                        qs = qsl if shared else qslb
                        ps_ = pssc.tile([P, TQ], f32, tag="psc", name="psc")
                        nc.tensor.matmul(
                            ps_[:, :w_ps],
                            kTn[:, h, c * P : (c + 1) * P],
                            qTn[:, h, qs],
                            start=True,
                            stop=False,
                        )
                        nc.tensor.matmul(
                            ps_[:, :w_ps],
                            kpe2[hb : hb + 64, c * P : (c + 1) * P],
                            qTr[hb : hb + 64, h // 2, qs],
                            start=False,
                            stop=True,
                        )
                        k = c - 4 * t if shared else c - nsh
                        if k >= 0:
                            w_ = (k + 1) * P
                            nc.vector.tensor_tensor(
                                ps_[:, :w_],
                                ps_[:, :w_],
                                cm_sb[:, (3 - k) * P : (3 - k) * P + w_],
                                ALU.add,
                            )
                        ex = wrkE.tile([P, TQ], bf16, tag="exp", name="ex")
                        nc.scalar.activation(
                            ex[:, :w_ps],
                            ps_[:, :w_ps],
                            AF.Exp,
                            bias=kb_sb[:, c : c + 1],
                            scale=SCALE,
                        )
                        exs.append(ex)
                    af = afp.tile([P, TQ], f32, tag="af", name="af")
                    for ui in range(2):
                        if ui == 0:
                            sls = [(exs[c][:, 0:256], c) for c in range(nsh)]
                        else:
                            sls = (
                                [(exs[c][:, 256:512], c) for c in range(nsh)]
                                + [(exs[nsh + m][:, 0:256], nsh + m)
                                   for m in range(2)]
                            )
                        pd = pspd.tile([P, TQ], f32, tag="pd", name="pd")
                        pav = psum.tile([P, TQ], f32, tag="mm", name="pav")
                        nlast = len(sls) - 1
                        for i, (exsl, c) in enumerate(sls):
                            nc.tensor.matmul(
                                pd[:, 0:256],
                                ones_sb[:],
                                exsl,
                                start=(i == 0),
                                stop=(i == nlast),
                            )
                            nc.tensor.matmul(
                                pav[:, 0:256],
                                vq[h // 4][
                                    :, c, (h % 4) * P : (h % 4 + 1) * P
                                ],
                                exsl,
                                start=(i == 0),
                                stop=(i == nlast),
                            )
                        rec = recp.tile([P, TQ], f32, tag="rec", name="rec")
                        with nc.allow_low_precision("softmax denom"):
                            nc.vector.reciprocal(rec[:, 0:256], pd[:, 0:256])
                        usl = slice(ui * 256, (ui + 1) * 256)
                        nc.vector.tensor_tensor(
                            af[:, usl], pav[:, 0:256], rec[:, 0:256], ALU.mult
                        )
                    # attnF = 16*attn; split to fp8
                    nc.scalar.copy(a8h[:, h, :], af[:])
                    nc.vector.tensor_tensor(
                        a8l[:, h, :], af[:], a8h[:, h, :], ALU.subtract
                    )

                def f_block(t, nt):
                    # output projection tile: split fp8, psum = 4096*out
                    a8h, a8l = a8[t]
                    wt = wop.tile([P, 2, 4, 2, TQ], f8, tag="wo", name="wo")
                    nc.sync.dma_start(wt[:], wo8[nt, :, :, :, :, :])
                    woh, wol = wt[:, 0, :, :, :], wt[:, 1, :, :, :]
                    for tqc in range(TQ // P):
                        po = pssc.tile([P, TQ], f32, tag="psc", name="po")
                        for hp in range(4):
                            hsl = slice(2 * hp, 2 * hp + 2)
                            csl = slice(tqc * P, (tqc + 1) * P)
                            for pi, (aop, wop_) in enumerate((
                                (a8h, woh),
                                (a8h, wol),
                                (a8l, woh),
                            )):
                                nc.tensor.matmul(
                                    po[:],
                                    aop[:, hsl, csl],
                                    wop_[:, hp, :, :],
                                    start=(hp == 0 and pi == 0),
                                    stop=(hp == 3 and pi == 2),
                                    perf_mode=DR,
                                )
                        ot = outp.tile([P, TQ], mybir.dt.bfloat16, tag="osb",
                                       name="ot")
                        if (nt + tqc) % 2 == 0:
                            nc.vector.tensor_copy(ot[:], po[:])
                        else:
                            nc.scalar.copy(ot[:], po[:])
                        nc.sync.dma_start(out[t, nt, tqc, :, :], ot[:])

                # t=1 attention, then t=0 attention interleaved with t=1's
                # output projection (PE queue alternates so neither phase's
                # dependency stalls leave the PE idle), then t=0's output
                for h in range(HG):
                    attn_head(1, h)
                STAGE_MARKS.append(("EFi", nc.next_id()))
                for h in range(HG):
                    attn_head(0, h)
                    f_block(1, h)
                STAGE_MARKS.append(("EFt", nc.next_id()))
                for nt in range(NHID):
                    f_block(0, nt)

    nc.finalize()
    return nc


def _get_program():
    if "nc" not in _CACHED:
        _CACHED["nc"] = _build_program()
    return _CACHED["nc"]


def _split8(a, scale):
    """a*scale split into same-scale fp8e4m3 hi + lo."""
    import ml_dtypes

    f8 = ml_dtypes.float8_e4m3
    s = np.asarray(a, np.float32) * np.float32(scale)
    hi = s.astype(f8)
    lo = (s - hi.astype(np.float32)).astype(f8)
    return hi, lo


def _host_prep(x, wq_a, q_norm_w, wq_b, wkv_a, kv_norm_w, wkv_b, wo,
               attention_mask, positions):
    """Build the 8 per-core input maps.

    All weight tensors are host-packed partition-major so every device
    DMA is one large contiguous transfer.
    """
    import ml_dtypes

    f = np.float32
    bf = ml_dtypes.bfloat16
    f8 = ml_dtypes.float8_e4m3
    x = np.asarray(x, f)
    wq_a = np.asarray(wq_a, f)
    wkv_a = np.asarray(wkv_a, f)
    # fold RMSNorm weights into the up-projections
    wq_b3 = (np.asarray(wq_b, f)
             * np.asarray(q_norm_w, f)[:, None]).reshape(Q_RANK, H, D_QK)
    wkv_b3 = (np.asarray(wkv_b, f)
              * np.asarray(kv_norm_w, f)[:, None]).reshape(
                  KV_RANK, H, D_NOPE + D_V)
    wo2 = np.asarray(wo, f)
    attention_mask = np.asarray(attention_mask)
    positions = np.asarray(positions)

    # stage-A weights: [chunk, hi/lo, p, pair, slot, width]
    def packA(w, nch):
        # w: [HID, nch*width] -> [nch, P(row), 2(hi/lo), KP, 2, width]
        width = w.shape[1] // nch
        hi, lo = _split8(w, SWA)
        def tr(a):
            return a.reshape(KP, 2, P, nch, width).transpose(3, 2, 0, 1, 4)
        return np.ascontiguousarray(
            np.stack([tr(hi), tr(lo)], axis=2))

    wq8 = packA(wq_a, QCH)
    wkv8 = packA(wkv_a[:, :KV_RANK], KCH)
    wr8 = packA(wkv_a[:, KV_RANK:], 1)[0]
    assert wr8.shape == (P, 2, KP, 2, 64), wr8.shape

    inv_freq = 1.0 / (THETA ** (np.arange(0, D_ROPE, 2, dtype=np.float64) / D_ROPE))

    # sliding causal mask: cm[dk, u] = 0 iff dk <= u - 384
    dk = np.arange(P)[:, None]
    u = np.arange(7 * P)[None, :]
    cmask = np.where(dk <= u - 3 * P, 0.0, NMASK).astype(f)
    onesd = np.full((P, P), 1.0 / 16.0, ml_dtypes.bfloat16)
    ones8d = np.full((P, 2 * P), 1.0 / 16.0, f8)

    per_batch = {}
    for b in range(B):
        xT = np.ascontiguousarray(x[b].T)  # [HID, T]
        xhi, xlo = _split8(xT, SX)
        def trx(a):
            # [HID, T] -> [4(grp), P, 4(pair), 2(slot), T]
            return (a.reshape(4, 4, 2, P, T)
                    .transpose(0, 3, 1, 2, 4))
        xhl = np.ascontiguousarray(np.stack([trx(xhi), trx(xlo)], axis=0))
        ang = positions[b].astype(np.float64)[:, None] * inv_freq[None, :]
        cosT = np.cos(ang).astype(f).T  # [32, T]
        sinT = np.sin(ang).astype(f).T
        cos4 = np.ascontiguousarray(np.tile(cosT, (4, 1)).astype(bf))
        sin4 = np.ascontiguousarray(
            np.concatenate([-sinT, sinT, -sinT, sinT], axis=0).astype(bf))
        kb = np.where(attention_mask[b] != 0, 0.0, NMASK).astype(f)
        kbias = np.ascontiguousarray(kb.reshape(TC, P).T)
        per_batch[b] = (xhl, cos4, sin4, kbias)

    in_maps = []
    for c in range(8):
        b, g = c // 4, c % 4
        hs = slice(g * HG, (g + 1) * HG)
        xhl, cos4, sin4, kbias = per_batch[b]
        # [rank, head, dim] -> [h, p, k, c] / [pair, p, k, c]
        wqbn_ = np.ascontiguousarray(
            wq_b3[:, hs, :D_NOPE]
            .reshape(QCH, P, HG, P).transpose(2, 1, 0, 3).astype(bf))
        # rope cols packed in head pairs: [h_even 64 | h_odd 64] per 128-col
        wqbr_ = np.ascontiguousarray(
            wq_b3[:, hs, D_NOPE:]
            .reshape(QCH, P, HG // 2, P).transpose(2, 1, 0, 3).astype(bf))
        wkbn_ = np.ascontiguousarray(
            wkv_b3[:, hs, :D_NOPE]
            .reshape(KCH, P, HG, P).transpose(1, 2, 0, 3).astype(bf))
        wkbv_ = np.ascontiguousarray(
            wkv_b3[:, hs, D_NOPE:]
            .reshape(KCH, P, 2, TQ).transpose(2, 1, 0, 3).astype(bf))
        # wo rows h*128+dv, cols nt*512+c
        #  -> [hl, nt, dv(p), hpair(4), h%2, c]
        wsh = wo2[g * HG * D_V : (g + 1) * HG * D_V, :]
        whi, wlo = _split8(wsh, SWO)
        def trwo(a):
            return a.reshape(4, 2, P, NHID, TQ).transpose(3, 2, 0, 1, 4)
        wo8 = np.ascontiguousarray(
            np.stack([trwo(whi), trwo(wlo)], axis=2))
        in_maps.append({
            "xhl": xhl, "wq8": wq8, "wkv8": wkv8, "wr8": wr8,
            "wqbn": wqbn_, "wqbr": wqbr_, "wkbn": wkbn_, "wkbv": wkbv_,
            "wo8": wo8,
            "cos4": cos4, "sin4": sin4,
            "cmask": cmask, "kbias": kbias,
            "onesd": onesd, "ones8d": ones8d,
        })
    return in_maps


def kernel(**inputs):
    from concourse.bass_utils import run_bass_kernel_spmd

    nc = _get_program()
    in_maps = _host_prep(**inputs)
    res = run_bass_kernel_spmd(nc, in_maps, core_ids=list(range(8)))
    _CACHED["last_result"] = res
    out = np.zeros((B, T, HID), np.float32)
    for c in range(8):
        blk = np.asarray(res.results[c]["out"], np.float32)
        # row = t*512 + q*128 + p, col = nt*512 + c; psum carries 4096*out
        out[c // 4] += blk.transpose(0, 2, 3, 1, 4).reshape(T, HID)
    out *= np.float32(1.0 / OSC)
    return out


# revision 35
# speedup vs baseline: 1.0246x; 1.0246x over previous
"""DeepseekV3 MLA forward on 8 TRN2 NeuronCores.

Sharding: data-parallel over batch (B=2 -> 2 groups of 4 cores), tensor-
parallel over heads within each batch group (32 heads -> 4 groups of 8).
Each core computes its batch element's full latent projections (wq_a /
wkv_a replicated), its 8 heads' q/k/v expansions + attention, and a
partial output projection (wo row-shard); the host sums the 4 partial
outputs per batch element.

Precision strategy (rel-err budget 2e-2; measured ~2e-3):
  - Stage A (x @ wq_a / wkv_a) runs in SPLIT fp8e4m3 with DoubleRow
    matmuls: both operands are host tensors, so the host provides
    hi = Q(s*v) and lo = Q(s*v - hi) at the SAME scale. The product is
    hh + hl + lh (lo*lo dropped) = 3 slot-products = 1.5 DoubleRow
    instructions per pair of 128-contraction tiles -> 0.75 cycles/row
    vs 1.0 for f32r, with ~bf16-level accuracy.
  - rms sum-of-squares: fp8 squares + fp8-DoubleRow ones-matmul.
  - Output projection (attn @ wo) in split fp8 as well: attnF = 16*attn
    (f32) -> hi (ACT copy) + lo (DVE subtract), wo hi/lo from host.
    PSUM result = 4096*out, DMA'd straight from PSUM to DRAM; the host
    descales by 1/4096 while summing the 4 partials.
  - Everything else (q/kv up-proj, scores, softmax, attn*v) stays f32r.
  - RMSNorm weights are folded into wq_b/wkv_b rows on the host.

Dataflow on device keeps activations transposed ([feature, token]) so
every matmul contracts over the partition dim with no on-device
transposes anywhere (see per-stage comments).  The softmax denominator
uses a ones/16-matmul so attnF lands pre-scaled by 16 for fp8.
All f32r matmuls are FP22-truncated fp32, single pass.
"""

import os
import sys

import numpy as np

sys.path.insert(0, "/opt/trn_rl_repo")

B, T, HID = 2, 1024, 4096
H, D_NOPE, D_ROPE, D_V = 32, 128, 64, 128
D_QK = D_NOPE + D_ROPE
Q_RANK, KV_RANK = 1536, 512
THETA, EPS = 10000.0, 1e-6
SCALE = float(D_QK) ** -0.5
NMASK = -30000.0

HG = H // 4          # heads per core = 8
P = 128
QCH = Q_RANK // P    # 12 latent chunks (q)
KCH = KV_RANK // P   # 4 latent chunks (kv)
KP = HID // 256      # 16 contraction pairs for stage A
TQ = 512             # token tile (free dim) for most matmuls
NT = T // TQ         # 2 token tiles
TC = T // P          # 8 token chunks of 128
NHID = HID // TQ     # 8 output column tiles

SX = 32.0            # x fp8 scale
SWA = 2048.0         # wq_a/wkv_a fp8 scale
LS = SX * SWA        # stage-A psum scale
SAT = 16.0           # attn fp8 scale (from ones=1/16 denominator)
SWO = 256.0          # wo fp8 scale
OSC = SAT * SWO      # output psum descale (host side)

_CACHED = {}
STAGE_MARKS = []


def _build_program():
    import contextlib

    import concourse.bacc as bacc
    import concourse.mybir as mybir
    import concourse.tile as tile

    f32 = mybir.dt.float32
    f32r = mybir.dt.float32r
    bf16 = mybir.dt.bfloat16
    f8 = mybir.dt.float8e4
    AF = mybir.ActivationFunctionType
    ALU = mybir.AluOpType
    DR = mybir.MatmulPerfMode.DoubleRow

    nc = bacc.Bacc()

    # ---- DRAM I/O (per-core shapes; SPMD across the 8 cores) ----
    # stage-A operands host-split into same-scale fp8 hi/lo pairs, packed
    # partition-major with the DoubleRow slot dim adjacent
    xhl = nc.dram_tensor("xhl", (2, 4, P, 4, 2, T), f8, kind="ExternalInput")
    wq8 = nc.dram_tensor("wq8", (QCH, P, 2, KP, 2, P), f8, kind="ExternalInput")
    wkv8 = nc.dram_tensor("wkv8", (KCH, P, 2, KP, 2, P), f8, kind="ExternalInput")
    wr8 = nc.dram_tensor("wr8", (P, 2, KP, 2, 64), f8, kind="ExternalInput")
    wqbn = nc.dram_tensor("wqbn", (HG, P, QCH, P), mybir.dt.bfloat16, kind="ExternalInput")
    wqbr = nc.dram_tensor("wqbr", (HG // 2, P, QCH, P), mybir.dt.bfloat16, kind="ExternalInput")
    wkbn = nc.dram_tensor("wkbn", (P, HG, KCH, P), mybir.dt.bfloat16, kind="ExternalInput")
    wkbv = nc.dram_tensor("wkbv", (2, P, KCH, TQ), mybir.dt.bfloat16, kind="ExternalInput")
    wo8 = nc.dram_tensor("wo8", (NHID, P, 2, 4, 2, TQ), f8, kind="ExternalInput")
    cos4 = nc.dram_tensor("cos4", (P, T), mybir.dt.bfloat16, kind="ExternalInput")
    sin4 = nc.dram_tensor("sin4", (P, T), mybir.dt.bfloat16, kind="ExternalInput")  # +-sin
    cmask = nc.dram_tensor("cmask", (P, 7 * P), f32, kind="ExternalInput")
    kbias = nc.dram_tensor("kbias", (P, TC), f32, kind="ExternalInput")
    onesd = nc.dram_tensor("onesd", (P, P), mybir.dt.bfloat16, kind="ExternalInput")  # 1/16
    ones8d = nc.dram_tensor("ones8d", (P, 2 * P), f8, kind="ExternalInput")
    out = nc.dram_tensor("out", (NT, NHID, 4, P, TQ), mybir.dt.bfloat16, kind="ExternalOutput")

    def r(ap):
        return ap.bitcast(f32r)

    with tile.TileContext(nc) as tc, contextlib.ExitStack() as rstack:
        with (
            tc.tile_pool(name="const", bufs=1) as const,
            tc.tile_pool(name="psmm", bufs=3, space="PSUM") as psum,
            tc.tile_pool(name="pspd", bufs=2, space="PSUM") as pspd,
            tc.tile_pool(name="pssc", bufs=3, space="PSUM") as pssc,
        ):
            # ---- constants (persistent; DMAs deferred past the first
            # stage-A tiles so they don't delay the first matmuls) ----
            ones_sb = const.tile([P, P], bf16, tag="ones")      # value 1/16
            ones8_sb = const.tile([P, 2, P], f8, tag="ones8")   # value 1/16
            cos_sb = const.tile([P, T], bf16, tag="cos")
            sin_sb = const.tile([P, T], bf16, tag="sin")
            kb_sb = const.tile([P, TC], f32, tag="kb")
            zero_b = const.tile([P, 1], f32, tag="zb")
            nc.vector.memset(zero_b[:], 0.0)
            eps_b = const.tile([P, 1], f32, tag="eb")
            nc.vector.memset(eps_b[:], EPS)

            def emit_const_dmas():
                nc.sync.dma_start(ones_sb[:], onesd[:, :])
                nc.sync.dma_start(ones8_sb[:], ones8d[:, :])
                nc.sync.dma_start(cos_sb[:], cos4[:, :])
                nc.sync.dma_start(sin_sb[:], sin4[:, :])
                nc.sync.dma_start(kb_sb[:], kbias[:, :])

            def rmsnorm(lat, nch, fan, sspool, sstag, wrk):
                # fp8 squares (scale 2 -> (2*lat)^2 <= ~121) + fp8-DR
                # ones/16 matmul: pd = sum(lat^2)/4
                for t in range(NT):
                    ssp = sspool.tile([P, TQ], f32, tag=sstag, name="ssp")
                    npr = nch // 2
                    for pr in range(npr):
                        sq = wrk.tile([P, 2, TQ], f8, tag="sq", name="sq")
                        for s in range(2):
                            nc.scalar.activation(
                                sq[:, s, :],
                                lat[2 * pr + s][:, t * TQ : (t + 1) * TQ],
                                AF.Square,
                                bias=zero_b[:],
                                scale=2.0,
                            )
                        nc.tensor.matmul(
                            ssp[:],
                            ones8_sb[:],
                            sq[:],
                            start=(pr == 0),
                            stop=(pr == npr - 1),
                            perf_mode=DR,
                        )
                    std = wrk.tile([P, TQ], f32, tag="std", name="std")
                    nc.scalar.activation(
                        std[:], ssp[:], AF.Sqrt, bias=eps_b[:], scale=4.0 / fan
                    )
                    rstd = wrk.tile([P, TQ], bf16, tag="rstd", name="rstd")
                    with nc.allow_low_precision("rmsnorm rstd"):
                        nc.vector.reciprocal(rstd[:], std[:])
                    for m in range(nch):
                        sl = lat[m][:, t * TQ : (t + 1) * TQ]
                        nc.gpsimd.tensor_tensor(sl, sl, rstd[:], ALU.mult)

            with tc.tile_pool(name="qlatp", bufs=1) as qlatp:
                qlat = [
                    qlatp.tile([P, T], bf16, tag=f"qlat{i}", name=f"qlat{i}")
                    for i in range(QCH)
                ]
                with tc.tile_pool(name="kvlatp", bufs=1) as kvlatp:
                    kvlat = [
                        kvlatp.tile([P, T], bf16, tag=f"kvlat{i}", name=f"kvlat{i}")
                        for i in range(KCH + 1)
                    ]
                    with (
                        tc.tile_pool(name="wkbnp", bufs=1) as wkbnp,
                        tc.tile_pool(name="wkbvp", bufs=1) as wkbvp,
                    ):
                        # stage-D weights: pool reserved up front; DMAs
                        # emitted mid-stage-A
                        wkn = wkbnp.tile(
                            [P, HG, KCH, P], bf16, tag="wkbn", name="wkbn"
                        )
                        wkvts = [
                            wkbvp.tile(
                                [P, KCH, TQ], bf16, tag=f"wkbv{quad}", name="wkbv"
                            )
                            for quad in range(2)
                        ]

                        def emit_dweight_dmas():
                            nc.sync.dma_start(wkn[:], wkbn[:, :, :, :])
                            for quad in range(2):
                                nc.sync.dma_start(
                                    wkvts[quad][:],
                                    wkbv[quad, :, :, :],
                                )

                        # kpe2 lives to the end (right side)
                        kpep = rstack.enter_context(
                            tc.tile_pool(name="kpep", bufs=1, side="right"))
                        kpe2 = kpep.tile([P, T], bf16, tag="kpe2")

                        # ---- stage A: latent projections, split fp8 ----
                        # mblocks: (dram, chunk index or None for rope,
                        #           width, dest tile)
                        mblocks = (
                            [(wkv8, m, P, kvlat[m]) for m in range(KCH)]
                            + [(wr8, None, 64, kvlat[KCH])]
                            + [(wq8, m, P, qlat[m]) for m in range(QCH)]
                        )

                        wrkA_cm = tc.tile_pool(name="wrkA", bufs=2)
                        wrkA = wrkA_cm.__enter__()
                        kswp_cm = tc.tile_pool(name="kswp", bufs=1)
                        kswp = kswp_cm.__enter__()
                        with (
                            tc.tile_pool(name="xk", bufs=8) as xkp,
                            tc.tile_pool(name="wA", bufs=3) as wAp,
                        ):
                            # first weight chunk DMA'd before the x stream
                            # so PE can start as soon as x group 0 lands
                            wt0 = wAp.tile([P, 2, KP, 2, P], f8, tag="wA",
                                           name="wA")
                            nc.sync.dma_start(
                                wt0[:], mblocks[0][0][mblocks[0][1],
                                                      :, :, :, :, :])
                            # first 3 chunks interleaved x-group-major so
                            # PE isn't starved while the x stream lands
                            NPRE = 3
                            pre_w = [wt0]
                            for mbi in range(1, NPRE):
                                wdram, blki, mw, dest = mblocks[mbi]
                                wt = wAp.tile(
                                    [P, 2, KP, 2, P], f8, tag="wA", name="wA")
                                nc.sync.dma_start(
                                    wt[:], wdram[blki, :, :, :, :, :])
                                pre_w.append(wt)
                            # x hi/lo tiles: 4 pairs per DMA so the HWDGE
                            # issue cost doesn't delay the first weights
                            xg = [[None] * 4, [None] * 4]
                            for g in range(4):
                                for hl in range(2):
                                    xt_ = xkp.tile(
                                        [P, 4, 2, T], f8, tag="xk", name="xk"
                                    )
                                    nc.sync.dma_start(
                                        xt_[:], xhl[hl, g, :, :, :, :]
                                    )
                                    xg[hl][g] = xt_
                            pre_ps = {}
                            pre_pool = [psum, pssc, pspd]
                            pre_tag = ["mm", "psc", "pd"]
                            for mbi in range(NPRE):
                                for t in range(NT):
                                    pre_ps[(mbi, t)] = pre_pool[mbi].tile(
                                        [P, TQ], f32, tag=pre_tag[mbi],
                                        name="psA")
                            for g in range(4):
                                for mbi in range(NPRE):
                                    for t in range(NT):
                                        ps_ = pre_ps[(mbi, t)]
                                        tsl = slice(t * TQ, (t + 1) * TQ)
                                        for j in range(4):
                                            pair = g * 4 + j
                                            for pi, (whl, xhl_) in enumerate((
                                                (0, 0), (1, 0), (0, 1),
                                            )):
                                                nc.tensor.matmul(
                                                    ps_[:, :],
                                                    pre_w[mbi][:, whl, pair,
                                                               :, :],
                                                    xg[xhl_][g][:, j, :, tsl],
                                                    start=(pair == 0
                                                           and pi == 0),
                                                    stop=(pair == KP - 1
                                                          and pi == 2),
                                                    perf_mode=DR,
                                                )
                            for mbi in range(NPRE):
                                wdram, blki, mw, dest = mblocks[mbi]
                                for t in range(NT):
                                    tsl = slice(t * TQ, (t + 1) * TQ)
                                    nc.scalar.activation(
                                        dest[:, tsl],
                                        pre_ps[(mbi, t)][:],
                                        AF.Copy,
                                        scale=1.0 / LS,
                                    )
                            emit_const_dmas()
                            for mbi, (wdram, blki, mw, dest) in \
                                    enumerate(mblocks):
                                if mbi < NPRE:
                                    continue
                                if mbi == NPRE:
                                    emit_dweight_dmas()
                                wt = wAp.tile(
                                    [P, 2, KP, 2, P], f8, tag="wA",
                                    name="wA",
                                )
                                if blki is None:
                                    nc.sync.dma_start(
                                        wt[:, :, :, :, :mw],
                                        wdram[:, :, :, :, :])
                                else:
                                    nc.sync.dma_start(
                                        wt[:], wdram[blki, :, :, :, :, :])
                                for t in range(NT):
                                    pp, ptag = (
                                        (psum, "mm")
                                        if (mbi + t) % 2 == 0
                                        else (pssc, "psc")
                                    )
                                    ps_ = pp.tile(
                                        [P, TQ], f32, tag=ptag, name="psA"
                                    )
                                    tsl = slice(t * TQ, (t + 1) * TQ)
                                    for pair in range(KP):
                                        g, j = pair // 4, pair % 4
                                        for pi, (whl, xhl_) in enumerate((
                                            (0, 0), (1, 0), (0, 1),
                                        )):
                                            nc.tensor.matmul(
                                                ps_[:mw, :],
                                                wt[:, whl, pair, :, :mw],
                                                xg[xhl_][g][:, j, :, tsl],
                                                start=(pair == 0 and pi == 0),
                                                stop=(pair == KP - 1
                                                      and pi == 2),
                                                perf_mode=DR,
                                            )
                                    nc.scalar.activation(
                                        dest[:mw, tsl],
                                        ps_[:mw, :],
                                        AF.Copy,
                                        scale=1.0 / LS,
                                    )
                                if mbi == KCH:
                                    # kv latents + rope chunk done: norm +
                                    # k rope now, overlapping the q blocks
                                    STAGE_MARKS.append(("A2kv", nc.next_id()))
                                    rmsnorm(kvlat, KCH, KV_RANK,
                                            pspd, "pd", wrkA)
                                    ksw = kswp.tile(
                                        [P, T], bf16, tag="ksw", name="ksw"
                                    )
                                    kswf = ksw[:]
                                    nc.sync.dma_start(
                                        kpe2[0:64, :], kvlat[KCH][0:64, :])
                                    nc.sync.dma_start(
                                        kpe2[64:128, :], kvlat[KCH][0:64, :])
                                    nc.sync.dma_start(
                                        ksw[0:32, :], kvlat[KCH][32:64, :])
                                    nc.sync.dma_start(
                                        ksw[32:64, :], kvlat[KCH][0:32, :])
                                    nc.sync.dma_start(
                                        ksw[64:96, :], kvlat[KCH][32:64, :])
                                    nc.sync.dma_start(
                                        ksw[96:128, :], kvlat[KCH][0:32, :])
                                    nc.vector.tensor_tensor(
                                        kswf, kswf, sin_sb[:], ALU.mult)
                                    nc.vector.tensor_tensor(
                                        kpe2[:], kpe2[:], cos_sb[:], ALU.mult)
                                    nc.vector.tensor_tensor(
                                        kpe2[:], kpe2[:], kswf, ALU.add)

                        # x + stage-A weight pools closed: SBUF freed for
                        # the kTn / vq activation tiles
                        wqbp_e = rstack.enter_context(
                            tc.tile_pool(name="wqbpe", bufs=4, side="right"))
                        wqb_pre = {}
                        for p__ in range(3):
                            wt_ = wqbp_e.tile(
                                [P, QCH, P], bf16, tag="wqb", name="wqbr")
                            nc.sync.dma_start(wt_[:], wqbr[p__, :, :, :])
                            wqb_pre[p__] = wt_
                        kTnp = rstack.enter_context(
                            tc.tile_pool(name="kTnp", bufs=1, side="right"))
                        kTn = kTnp.tile([P, HG, T], bf16, tag="kTn")
                        vqp = rstack.enter_context(
                            tc.tile_pool(name="vqp", bufs=2, side="right"))
                        vq = [
                            vqp.tile([P, TC, 4 * D_V], bf16, tag="vq", name="vq")
                            for _ in range(2)
                        ]

                        STAGE_MARKS.append(("A2q", nc.next_id()))
                        rmsnorm(qlat, QCH, Q_RANK, pssc, "psc", wrkA)
                        kswp_cm.__exit__(None, None, None)
                        wrkA_cm.__exit__(None, None, None)
                        STAGE_MARKS.append(("D", nc.next_id()))
                        # ---- stage D: kT_nope per head, v per quad ----
                        for h in range(HG):
                            pp, ptag = (psum, "mm") if h % 2 == 0 else (pssc, "psc")
                            pst = [
                                pp.tile([P, TQ], f32, tag=ptag, name="psD")
                                for _ in range(NT)
                            ]
                            for k in range(KCH):
                                for t in range(NT):
                                    nc.tensor.matmul(
                                        pst[t][:],
                                        wkn[:, h, k, :],
                                        kvlat[k][:, t * TQ : (t + 1) * TQ],
                                        start=(k == 0),
                                        stop=(k == KCH - 1),
                                    )
                            for t in range(NT):
                                nc.scalar.copy(
                                    kTn[:, h, t * TQ : (t + 1) * TQ], pst[t][:]
                                )
                        for quad in range(2):
                            for tkc in range(TC):
                                pp, ptag = (
                                    (psum, "mm") if tkc % 2 == 0 else (pssc, "psc")
                                )
                                ps_ = pp.tile([P, TQ], f32, tag=ptag, name="psV")
                                for k in range(KCH):
                                    nc.tensor.matmul(
                                        ps_[:],
                                        kvlat[k][:, tkc * P : (tkc + 1) * P],
                                        wkvts[quad][:, k, :],
                                        start=(k == 0),
                                        stop=(k == KCH - 1),
                                    )
                                if tkc % 2 == 0:
                                    nc.vector.tensor_copy(
                                        vq[quad][:, tkc, :], ps_[:])
                                else:
                                    nc.scalar.copy(
                                        vq[quad][:, tkc, :], ps_[:])


                # kvlat + stage-D weight pools closed here
                actq = rstack.enter_context(
                    tc.tile_pool(name="actq", bufs=1, side="right"))
                qTn = actq.tile([P, HG, T], bf16, tag="qTn")
                qTr = actq.tile([P, HG // 2, T], bf16, tag="qTr")

                cmp_ = rstack.enter_context(
                    tc.tile_pool(name="cmp", bufs=1, side="right"))
                cm_sb = cmp_.tile([P, 7 * P], f32, tag="cm")
                nc.sync.dma_start(cm_sb[:], cmask[:, :])
                STAGE_MARKS.append(("B", nc.next_id()))
                # ---- stage B: qT per head (one DMA per head/pair) ----
                with (
                    tc.tile_pool(name="qswp", bufs=1) as qswp,
                ):
                    wqbp = wqbp_e
                    for p_ in range(HG // 2):
                        if p_ in wqb_pre:
                            wt = wqb_pre[p_]
                        else:
                            wt = wqbp.tile(
                                [P, QCH, P], bf16, tag="wqb", name="wqbr")
                            nc.sync.dma_start(wt[:], wqbr[p_, :, :, :])
                        pp, ptag = (psum, "mm") if p_ % 2 == 0 else (pssc, "psc")
                        pst = [
                            pp.tile([P, TQ], f32, tag=ptag, name="psB2")
                            for _ in range(NT)
                        ]
                        for k in range(QCH):
                            for t in range(NT):
                                nc.tensor.matmul(
                                    pst[t][:],
                                    wt[:, k, :],
                                    qlat[k][:, t * TQ : (t + 1) * TQ],
                                    start=(k == 0),
                                    stop=(k == QCH - 1),
                                )
                        for t in range(NT):
                            nc.scalar.copy(
                                qTr[:, p_, t * TQ : (t + 1) * TQ], pst[t][:]
                            )
                        # rope this pair immediately (overlaps next pair)
                        qsw = qswp.tile([P, T], bf16, tag="qsw", name="qsw")
                        qp = qTr[:, p_, :]
                        nc.sync.dma_start(qsw[0:32, :], qp[32:64, :])
                        nc.sync.dma_start(qsw[32:64, :], qp[0:32, :])
                        nc.sync.dma_start(qsw[64:96, :], qp[96:128, :])
                        nc.sync.dma_start(qsw[96:128, :], qp[64:96, :])
                        nc.vector.tensor_tensor(qsw[:], qsw[:], sin_sb[:], ALU.mult)
                        nc.vector.tensor_tensor(qp, qp, cos_sb[:], ALU.mult)
                        nc.vector.tensor_tensor(qp, qp, qsw[:], ALU.add)
                    for h in range(HG):
                        wt = wqbp.tile(
                            [P, QCH, P], bf16, tag="wqb", name="wqbn")
                        nc.sync.dma_start(wt[:], wqbn[h, :, :, :])
                        pp, ptag = (psum, "mm") if h % 2 == 0 else (pssc, "psc")
                        pst = [
                            pp.tile([P, TQ], f32, tag=ptag, name="psB")
                            for _ in range(NT)
                        ]
                        for k in range(QCH):
                            for t in range(NT):
                                nc.tensor.matmul(
                                    pst[t][:],
                                    wt[:, k, :],
                                    qlat[k][:, t * TQ : (t + 1) * TQ],
                                    start=(k == 0),
                                    stop=(k == QCH - 1),
                                )
                        for t in range(NT):
                            nc.scalar.copy(
                                qTn[:, h, t * TQ : (t + 1) * TQ], pst[t][:]
                            )
            STAGE_MARKS.append(("EF", nc.next_id()))
            # kvlat + qlat pools closed here
            # ---- stages E+F per token tile (t=1 first: its leading tk
            # chunks need no causal mask, hiding the mask DMA) ----
            with (
                tc.tile_pool(name="attp", bufs=1) as attp,
                tc.tile_pool(name="wrkE", bufs=3) as wrkE,
                tc.tile_pool(name="recp", bufs=2) as recp,
                tc.tile_pool(name="afp", bufs=3) as afp,
                tc.tile_pool(name="wop", bufs=3) as wop,
                tc.tile_pool(name="outs", bufs=4) as outp,
            ):
                a8 = {}
                for t in (1, 0):
                    # attn in split fp8: hi/lo tiles [dv, head, tq]
                    a8[t] = (
                        attp.tile([P, HG, TQ], f8, tag=f"a8h{t}", name="a8h"),
                        attp.tile([P, HG, TQ], f8, tag=f"a8l{t}", name="a8l"),
                    )

                def attn_head(t, h):
                    # 256-query sub-tiling: query halves ua=2t, ub=2t+1.
                    # Key chunks 0..4t+1 are shared (both halves, 512-wide
                    # matmuls); chunks 4t+2, 4t+3 touch only ub (256-wide).
                    # Masked work drops ~25% vs 512-query tiling.
                    a8h, a8l = a8[t]
                    hb = 64 * (h % 2)
                    qsl = slice(t * TQ, (t + 1) * TQ)
                    qslb = slice(t * TQ + 256, (t + 1) * TQ)
                    nsh = 4 * t + 2
                    exs = []
                    for c in range(nsh + 2):
                        shared = c < nsh
                        w_ps = TQ if shared else 256
                        qs = qsl if shared else qslb
                        ps_ = pssc.tile([P, TQ], f32, tag="psc", name="psc")
                        nc.tensor.matmul(
                            ps_[:, :w_ps],
                            kTn[:, h, c * P : (c + 1) * P],
                            qTn[:, h, qs],
                            start=True,
                            stop=False,
                        )
                        nc.tensor.matmul(
                            ps_[:, :w_ps],
                            kpe2[hb : hb + 64, c * P : (c + 1) * P],
                            qTr[hb : hb + 64, h // 2, qs],
                            start=False,
                            stop=True,
                        )
                        k = c - 4 * t if shared else c - nsh
                        if k >= 0:
                            w_ = (k + 1) * P
                            nc.vector.tensor_tensor(
                                ps_[:, :w_],
                                ps_[:, :w_],
                                cm_sb[:, (3 - k) * P : (3 - k) * P + w_],
                                ALU.add,
                            )
                        ex = wrkE.tile([P, TQ], bf16, tag="exp", name="ex")
                        nc.scalar.activation(
                            ex[:, :w_ps],
                            ps_[:, :w_ps],
                            AF.Exp,
                            bias=kb_sb[:, c : c + 1],
                            scale=SCALE,
                        )
                        exs.append(ex)
                    af = afp.tile([P, TQ], f32, tag="af", name="af")
                    for ui in range(2):
                        if ui == 0:
                            sls = [(exs[c][:, 0:256], c) for c in range(nsh)]
                        else:
                            sls = (
                                [(exs[c][:, 256:512], c) for c in range(nsh)]
                                + [(exs[nsh + m][:, 0:256], nsh + m)
                                   for m in range(2)]
                            )
                        pd = pspd.tile([P, TQ], f32, tag="pd", name="pd")
                        pav = psum.tile([P, TQ], f32, tag="mm", name="pav")
                        nlast = len(sls) - 1
                        for i, (exsl, c) in enumerate(sls):
                            nc.tensor.matmul(
                                pd[:, 0:256],
                                ones_sb[:],
                                exsl,
                                start=(i == 0),
                                stop=(i == nlast),
                            )
                            nc.tensor.matmul(
                                pav[:, 0:256],
                                vq[h // 4][
                                    :, c, (h % 4) * P : (h % 4 + 1) * P
                                ],
                                exsl,
                                start=(i == 0),
                                stop=(i == nlast),
                            )
                        rec = recp.tile([P, TQ], f32, tag="rec", name="rec")
                        with nc.allow_low_precision("softmax denom"):
                            nc.vector.reciprocal(rec[:, 0:256], pd[:, 0:256])
                        usl = slice(ui * 256, (ui + 1) * 256)
                        nc.vector.tensor_tensor(
                            af[:, usl], pav[:, 0:256], rec[:, 0:256], ALU.mult
                        )
                    # attnF = 16*attn; split to fp8
                    nc.scalar.copy(a8h[:, h, :], af[:])
                    nc.vector.tensor_tensor(
                        a8l[:, h, :], af[:], a8h[:, h, :], ALU.subtract
                    )

                def f_block(t, nt):
                    # output projection tile: split fp8, psum = 4096*out
                    a8h, a8l = a8[t]
                    wt = wop.tile([P, 2, 4, 2, TQ], f8, tag="wo", name="wo")
                    nc.sync.dma_start(wt[:], wo8[nt, :, :, :, :, :])
                    woh, wol = wt[:, 0, :, :, :], wt[:, 1, :, :, :]
                    for tqc in range(TQ // P):
                        po = pssc.tile([P, TQ], f32, tag="psc", name="po")
                        for hp in range(4):
                            hsl = slice(2 * hp, 2 * hp + 2)
                            csl = slice(tqc * P, (tqc + 1) * P)
                            for pi, (aop, wop_) in enumerate((
                                (a8h, woh),
                                (a8h, wol),
                                (a8l, woh),
                            )):
                                nc.tensor.matmul(
                                    po[:],
                                    aop[:, hsl, csl],
                                    wop_[:, hp, :, :],
                                    start=(hp == 0 and pi == 0),
                                    stop=(hp == 3 and pi == 2),
                                    perf_mode=DR,
                                )
                        ot = outp.tile([P, TQ], mybir.dt.bfloat16, tag="osb",
                                       name="ot")
                        if (nt + tqc) % 2 == 0:
                            nc.vector.tensor_copy(ot[:], po[:])
                        else:
                            nc.scalar.copy(ot[:], po[:])
                        nc.sync.dma_start(out[t, nt, tqc, :, :], ot[:])

                # t=1 attention, then t=0 attention interleaved with t=1's
                # output projection (PE queue alternates so neither phase's
                # dependency stalls leave the PE idle), then t=0's output
                for h in range(HG):
                    attn_head(1, h)
                STAGE_MARKS.append(("EFi", nc.next_id()))
                for h in range(HG):
                    attn_head(0, h)
                    f_block(1, h)
                STAGE_MARKS.append(("EFt", nc.next_id()))
                for nt in range(NHID):
                    f_block(0, nt)

    nc.finalize()
    return nc


def _get_program():
    if "nc" not in _CACHED:
        _CACHED["nc"] = _build_program()
    return _CACHED["nc"]


def _split8(a, scale):
    """a*scale split into same-scale fp8e4m3 hi + lo."""
    import ml_dtypes

    f8 = ml_dtypes.float8_e4m3
    s = np.asarray(a, np.float32) * np.float32(scale)
    hi = s.astype(f8)
    lo = (s - hi.astype(np.float32)).astype(f8)
    return hi, lo


def _host_prep(x, wq_a, q_norm_w, wq_b, wkv_a, kv_norm_w, wkv_b, wo,
               attention_mask, positions):
    """Build the 8 per-core input maps.

    All weight tensors are host-packed partition-major so every device
    DMA is one large contiguous transfer.
    """
    import ml_dtypes

    f = np.float32
    bf = ml_dtypes.bfloat16
    f8 = ml_dtypes.float8_e4m3
    x = np.asarray(x, f)
    wq_a = np.asarray(wq_a, f)
    wkv_a = np.asarray(wkv_a, f)
    # fold RMSNorm weights into the up-projections
    wq_b3 = (np.asarray(wq_b, f)
             * np.asarray(q_norm_w, f)[:, None]).reshape(Q_RANK, H, D_QK)
    wkv_b3 = (np.asarray(wkv_b, f)
              * np.asarray(kv_norm_w, f)[:, None]).reshape(
                  KV_RANK, H, D_NOPE + D_V)
    wo2 = np.asarray(wo, f)
    attention_mask = np.asarray(attention_mask)
    positions = np.asarray(positions)

    # stage-A weights: [chunk, hi/lo, p, pair, slot, width]
    def packA(w, nch):
        # w: [HID, nch*width] -> [nch, P(row), 2(hi/lo), KP, 2, width]
        width = w.shape[1] // nch
        hi, lo = _split8(w, SWA)
        def tr(a):
            return a.reshape(KP, 2, P, nch, width).transpose(3, 2, 0, 1, 4)
        return np.ascontiguousarray(
            np.stack([tr(hi), tr(lo)], axis=2))

    wq8 = packA(wq_a, QCH)
    wkv8 = packA(wkv_a[:, :KV_RANK], KCH)
    wr8 = packA(wkv_a[:, KV_RANK:], 1)[0]
    assert wr8.shape == (P, 2, KP, 2, 64), wr8.shape

    inv_freq = 1.0 / (THETA ** (np.arange(0, D_ROPE, 2, dtype=np.float64) / D_ROPE))

    # sliding causal mask: cm[dk, u] = 0 iff dk <= u - 384
    dk = np.arange(P)[:, None]
    u = np.arange(7 * P)[None, :]
    cmask = np.where(dk <= u - 3 * P, 0.0, NMASK).astype(f)
    onesd = np.full((P, P), 1.0 / 16.0, ml_dtypes.bfloat16)
    ones8d = np.full((P, 2 * P), 1.0 / 16.0, f8)

    per_batch = {}
    for b in range(B):
        xT = np.ascontiguousarray(x[b].T)  # [HID, T]
        xhi, xlo = _split8(xT, SX)
        def trx(a):
            # [HID, T] -> [4(grp), P, 4(pair), 2(slot), T]
            return (a.reshape(4, 4, 2, P, T)
                    .transpose(0, 3, 1, 2, 4))
        xhl = np.ascontiguousarray(np.stack([trx(xhi), trx(xlo)], axis=0))
        ang = positions[b].astype(np.float64)[:, None] * inv_freq[None, :]
        cosT = np.cos(ang).astype(f).T  # [32, T]
        sinT = np.sin(ang).astype(f).T
        cos4 = np.ascontiguousarray(np.tile(cosT, (4, 1)).astype(bf))
        sin4 = np.ascontiguousarray(
            np.concatenate([-sinT, sinT, -sinT, sinT], axis=0).astype(bf))
        kb = np.where(attention_mask[b] != 0, 0.0, NMASK).astype(f)
        kbias = np.ascontiguousarray(kb.reshape(TC, P).T)
        per_batch[b] = (xhl, cos4, sin4, kbias)

    in_maps = []
    for c in range(8):
        b, g = c // 4, c % 4
        hs = slice(g * HG, (g + 1) * HG)
        xhl, cos4, sin4, kbias = per_batch[b]
        # [rank, head, dim] -> [h, p, k, c] / [pair, p, k, c]
        wqbn_ = np.ascontiguousarray(
            wq_b3[:, hs, :D_NOPE]
            .reshape(QCH, P, HG, P).transpose(2, 1, 0, 3).astype(bf))
        # rope cols packed in head pairs: [h_even 64 | h_odd 64] per 128-col
        wqbr_ = np.ascontiguousarray(
            wq_b3[:, hs, D_NOPE:]
            .reshape(QCH, P, HG // 2, P).transpose(2, 1, 0, 3).astype(bf))
        wkbn_ = np.ascontiguousarray(
            wkv_b3[:, hs, :D_NOPE]
            .reshape(KCH, P, HG, P).transpose(1, 2, 0, 3).astype(bf))
        wkbv_ = np.ascontiguousarray(
            wkv_b3[:, hs, D_NOPE:]
            .reshape(KCH, P, 2, TQ).transpose(2, 1, 0, 3).astype(bf))
        # wo rows h*128+dv, cols nt*512+c
        #  -> [hl, nt, dv(p), hpair(4), h%2, c]
        wsh = wo2[g * HG * D_V : (g + 1) * HG * D_V, :]
        whi, wlo = _split8(wsh, SWO)
        def trwo(a):
            return a.reshape(4, 2, P, NHID, TQ).transpose(3, 2, 0, 1, 4)
        wo8 = np.ascontiguousarray(
            np.stack([trwo(whi), trwo(wlo)], axis=2))
        in_maps.append({
            "xhl": xhl, "wq8": wq8, "wkv8": wkv8, "wr8": wr8,
            "wqbn": wqbn_, "wqbr": wqbr_, "wkbn": wkbn_, "wkbv": wkbv_,
            "wo8": wo8,
            "cos4": cos4, "sin4": sin4,
            "cmask": cmask, "kbias": kbias,
            "onesd": onesd, "ones8d": ones8d,
        })
    return in_maps


def kernel(**inputs):
    from concourse.bass_utils import run_bass_kernel_spmd

    nc = _get_program()
    in_maps = _host_prep(**inputs)
    res = run_bass_kernel_spmd(nc, in_maps, core_ids=list(range(8)))
    _CACHED["last_result"] = res
    out = np.zeros((B, T, HID), np.float32)
    for c in range(8):
        blk = np.asarray(res.results[c]["out"], np.float32)
        # row = t*512 + q*128 + p, col = nt*512 + c; psum carries 4096*out
        out[c // 4] += blk.transpose(0, 2, 3, 1, 4).reshape(T, HID)
    out *= np.float32(1.0 / OSC)
    return out
